# revision 13
# baseline (speedup 1.0000x reference)
"""Trainium2 Bass kernel for 2-layer GCN message passing (nn_Method_GCN).

Strategy (8-core SPMD, full inputs in / full output out):
  - Fold weights first: Z = X @ W.T, so per-edge gathers move 64-dim (L1) /
    16-dim (L2) rows instead of 128-dim.
  - Push per-edge symmetric norm into per-node scaling:
        agg_i = rs_i * (sum_{j in N(i)} Y_j + Y_i),  Y = Z * rs,
    rs = deg^-1/2, so messages are plain unweighted row sums.
  - Shard destination nodes across 8 cores (6250 each), degree-sorted within
    each core so 128-node blocks have uniform slot counts.
  - Per block: one dma_gather (InstDMAGatherAnt) per table-half pulls all
    neighbor rows into SBUF [128, S, F]; contiguous pairwise tree-adds on DVE
    reduce slots; per-partition scale+relu on ACT.
  - Tables are AllGather'd (bf16 Y1 / f32 Y2) with 256B row pitch; int16
    gather indices address two halves split at the core 0-3 | 4-7 boundary.
  - deg is computed on device from the slot-validity mask.
"""
import os
import sys
import time

import numpy as np

for _p in ("/opt/trn_rl_repo", "/root/.axon_site/_ro/trn_rl_repo"):
    if os.path.isdir(_p) and _p not in sys.path:
        sys.path.append(_p)

import ml_dtypes
import concourse.bass as bass
import concourse.bacc as bacc
import concourse.tile as tile
import concourse.mybir as mybir

P = 128
NCORES = 8
F1, F2, D = 64, 16, 128
PITCH1 = 128          # bf16 row pitch (256B) in Y1 table
PITCH2 = 64           # f32 row pitch (256B) in Y2 table
ZPAD = 16             # zero rows at head and tail of each table
MAX_GS = 144          # slot budget per gather group (stage tile sizing)

_CACHE = {}


# ----------------------------------------------------------------- raw gather
def _raw_dma_gather(gps, out_ap, in_ap, idxs_ap, num_idxs, elem_size, elem_step):
    """dma_gather without the elem_size%256B assert (HW needs only the row
    pitch to be a 256B multiple — verified on silicon)."""
    nc = gps.bass
    assert idxs_ap.dtype == mybir.dt.int16
    assert in_ap.dtype == out_ap.dtype
    stride_bytes = elem_step * mybir.dt.size(in_ap.dtype)
    assert stride_bytes % 256 == 0
    return gps.add_instruction(
        mybir.InstDMAGatherAnt(
            name=nc.get_next_instruction_name(),
            ins=[
                *gps.lower_ap_dma(in_ap, for_custom_bir_dma=True),
                gps.lower_ap(idxs_ap),
                gps.lower_val_access(gps.to_reg(num_idxs)),
            ],
            outs=[gps.lower_ap(out_ap)],
            transpose=False,
            num_idxs=num_idxs,
            elem_size=elem_size,
            stride_bytes_256=stride_bytes // 256,
            gen_mode=0,
            single_packet=True,
            queue_num=0,
            sbuf_tokens_per_rank=0,
            sbuf_free_dim_per_rank=0,
            sbuf_free_dim_pad_per_rank=0,
            sbuf_byte_offset=0,
        )
    )


def _calls(gs, k=8):
    """Split gs slot columns into per-call spans of <=k columns (<=128*k idx)."""
    out = []
    c0 = 0
    while c0 < gs:
        n = min(k, gs - c0)
        out.append((c0, n))
        c0 += n
    return out


def _fold(nc, stage, col0, width, fdim):
    """In-place pairwise tree-sum of `width` row-chunks of fdim cols starting
    at slot column col0; result lands in slot col0. Slices stay contiguous."""
    w = width
    while w > 1:
        h = w // 2
        lo = col0 * fdim
        nc.vector.tensor_tensor(
            out=stage[:, lo : lo + h * fdim],
            in0=stage[:, lo : lo + h * fdim],
            in1=stage[:, lo + (w - h) * fdim : lo + w * fdim],
            op=mybir.AluOpType.add,
        )
        w = w - h


# ------------------------------------------------------------- preprocessing
def _preprocess(edges, n_nodes):
    """Graph partitioning: per-core degree-sorted node permutation, padded
    per-block slot grids for both table halves, wrapped int16 gather indices,
    validity mask. Returns the SPMD-uniform structure + per-core arrays."""
    npc = n_nodes // NCORES
    nblk = (npc + P - 1) // P
    half = n_nodes // 2

    ed = np.asarray(edges).astype(np.int64, copy=False)
    src, dst = ed[:, 0], ed[:, 1]
    cnt_raw = np.bincount(src, minlength=n_nodes) + np.bincount(dst, minlength=n_nodes)

    # per-core degree-descending permutation
    gperm = np.empty(n_nodes, dtype=np.int64)
    for c in range(NCORES):
        ids = np.arange(c * npc, (c + 1) * npc)
        gperm[c * npc : (c + 1) * npc] = ids[np.argsort(-cnt_raw[ids], kind="stable")]
    gpos = np.empty(n_nodes, dtype=np.int64)
    gpos[gperm] = np.arange(n_nodes)

    # directed messages sorted by destination position; then self-appended
    mdst = gpos[np.concatenate([dst, src])]
    msrc = gpos[np.concatenate([src, dst])]
    order = np.argsort(mdst, kind="stable")
    mdst_s = mdst[order]
    msrc_s = msrc[order]
    bnd = np.searchsorted(mdst_s, np.arange(n_nodes + 1))
    lens = (bnd[1:] - bnd[:-1]).astype(np.int64)          # neighbors per position
    a_of = (msrc_s < half)

    # per-position A/B counts (+ self in its own half)
    nA = np.zeros(n_nodes, dtype=np.int64)
    np.add.at(nA, mdst_s, a_of)
    nB = lens - nA
    selfA = (np.arange(n_nodes) < half)
    nA = nA + selfA
    nB = nB + (~selfA)

    # SPMD-uniform slot counts per block (max over cores), even, >= 2
    SA = np.zeros(nblk, dtype=np.int64)
    SB = np.zeros(nblk, dtype=np.int64)
    for b in range(nblk):
        mA = mB = 0
        for c in range(NCORES):
            g0 = c * npc + b * P
            g1 = min(g0 + P, (c + 1) * npc)
            mA = max(mA, int(nA[g0:g1].max()))
            mB = max(mB, int(nB[g0:g1].max()))
        SA[b] = max(2, (mA + 1) // 2 * 2)
        SB[b] = max(2, (mB + 1) // 2 * 2)

    # group blocks under the stage-slot budget
    groups = []          # list of lists of block ids
    cur, cur_s = [], 0
    for b in range(nblk):
        s = int(SA[b] + SB[b])
        if cur and cur_s + s > MAX_GS:
            groups.append(cur)
            cur, cur_s = [], 0
        cur.append(b)
        cur_s += s
    if cur:
        groups.append(cur)

    gs_max = max(int(sum(SA[b] + SB[b] for b in g)) for g in groups)

    zrA = 0                     # zero row at table head (A-half local)
    zrB = half                  # zero row at table tail (B-half local: ZPAD+half+... )
    tot_s = int((SA + SB).sum())
    tot16 = 8 * tot_s

    # per-core fills
    cores = []
    for c in range(NCORES):
        idxw = np.zeros((16, tot16), dtype=np.int16)
        msk = np.zeros((P, tot_s), dtype=np.float32)
        moff = 0
        w16 = 0
        for g in groups:
            gsA = int(SA[list(g)].sum()) if hasattr(SA, "sum") else 0
            # build A-region and B-region slot grids for this group
            gsA = int(sum(SA[b] for b in g))
            gsB = int(sum(SB[b] for b in g))
            gridA = np.full((P, gsA), zrA, dtype=np.int64)
            gridB = np.full((P, gsB), zrB, dtype=np.int64)
            a0 = b0 = 0
            for b in g:
                g0 = c * npc + b * P
                g1 = min(g0 + P, (c + 1) * npc)
                rows = g1 - g0
                for p in range(rows):
                    gg = g0 + p
                    nb = msrc_s[bnd[gg] : bnd[gg + 1]]
                    nbA = nb[nb < half] + ZPAD
                    nbB = nb[nb >= half] - half
                    if gg < half:
                        nbA = np.append(nbA, gg + ZPAD)
                    else:
                        nbB = np.append(nbB, gg - half)
                    gridA[p, a0 : a0 + len(nbA)] = nbA
                    gridB[p, b0 : b0 + len(nbB)] = nbB
                    tot_real = len(nbA) + len(nbB)
                    msk[p, moff : moff + tot_real] = 1.0
                a0 += int(SA[b])
                b0 += int(SB[b])
                moff += int(SA[b] + SB[b])
            # wrapped int16 lists, one segment per gather call (<=1024 idx):
            # linear i = s*128 + p -> [i%16, i//16]
            for grid, gs in ((gridA, gsA), (gridB, gsB)):
                for c0, ncol in _calls(gs):
                    n = P * ncol
                    lin = grid[:, c0 : c0 + ncol].T.reshape(-1).astype(np.int16)
                    ii = np.arange(n)
                    wrap = np.zeros((16, n // 16), dtype=np.int16)
                    wrap[ii % 16, ii // 16] = lin
                    idxw[:, w16 : w16 + n // 16] = wrap
                    w16 += n // 16
        assert w16 == tot16 and moff == tot_s
        cores.append((np.tile(idxw, (8, 1)), msk))

    struct = {
        "nblk": nblk,
        "npc": npc,
        "half": half,
        "SA": SA,
        "SB": SB,
        "groups": groups,
        "tot_s": tot_s,
        "tot16": tot16,
        "gs_max": gs_max,
        "n_nodes": n_nodes,
    }
    return struct, cores, gperm


# ------------------------------------------------------------------ program
def _build_bass(st, stop_after=None):
    nblk, npc, half = st["nblk"], st["npc"], st["half"]
    n_nodes, tot_s, tot16 = st["n_nodes"], st["tot_s"], st["tot16"]
    SA, SB, groups = st["SA"], st["SB"], st["groups"]
    gs_max = st["gs_max"]
    ncols = nblk * P
    tbl_rows = n_nodes + 2 * ZPAD
    rowsA = half + ZPAD          # A-half view: rows [0, half+ZPAD)
    f32, bf16, i16 = mybir.dt.float32, mybir.dt.bfloat16, mybir.dt.int16

    nc = bacc.Bacc(None, target_bir_lowering=False, debug=False)
    xpt_in = nc.declare_dram_parameter("xpt", [D, ncols], f32, isOutput=False)
    idxw_in = nc.declare_dram_parameter("idxw", [P, tot16], i16, isOutput=False)
    msk_in = nc.declare_dram_parameter("msk", [P, tot_s], f32, isOutput=False)
    w1t_in = nc.declare_dram_parameter("w1t", [D, F1], f32, isOutput=False)
    w2t_in = nc.declare_dram_parameter("w2t", [F1, F2], bf16, isOutput=False)
    out_p = nc.declare_dram_parameter("out", [npc, F2], f32, isOutput=True)

    y1_slice = nc.dram_tensor("y1_slice", [npc, F1], bf16)
    y2_slice = nc.dram_tensor("y2_slice", [npc, F2], f32)
    y1_gath = nc.dram_tensor("y1_gath", [n_nodes, F1], bf16)
    y2_gath = nc.dram_tensor("y2_gath", [n_nodes, F2], f32)
    y1_tbl = nc.dram_tensor("y1_tbl", [tbl_rows, PITCH1], bf16)
    y2_tbl = nc.dram_tensor("y2_tbl", [tbl_rows, PITCH2], f32)

    with tile.TileContext(nc) as tc:
        with (
            tc.tile_pool(name="res", bufs=1) as res,
            tc.tile_pool(name="tmp", bufs=6) as tmp,
            tc.tile_pool(name="stage1", bufs=4) as stage1p,
            tc.tile_pool(name="stage2", bufs=3) as stage2p,
            tc.tile_pool(name="io", bufs=4) as iop,
            tc.tile_pool(name="ps", bufs=2, space="PSUM") as ps,
        ):
            xpt = res.tile([D, ncols], f32)
            nc.sync.dma_start(out=xpt[:], in_=xpt_in.ap())
            idxw = res.tile([P, tot16], i16)
            nc.sync.dma_start(out=idxw[:], in_=idxw_in.ap())
            msk = res.tile([P, tot_s], f32)
            nc.sync.dma_start(out=msk[:], in_=msk_in.ap())
            w1t = res.tile([D, F1], f32)
            nc.sync.dma_start(out=w1t[:], in_=w1t_in.ap())
            w2t = res.tile([F1, F2], bf16)
            nc.sync.dma_start(out=w2t[:], in_=w2t_in.ap())
            ident = res.tile([P, P], bf16)
            from concourse.masks import make_identity

            make_identity(nc, ident[:])
            rs_all = res.tile([P, nblk], f32)

            # zero rows (head+tail) of both tables
            z1 = res.tile([ZPAD, F1], bf16)
            nc.gpsimd.memset(z1[:], 0)
            nc.sync.dma_start(out=y1_tbl[0:ZPAD, :F1], in_=z1[:])
            nc.sync.dma_start(out=y1_tbl[n_nodes + ZPAD :, :F1], in_=z1[:])
            z2 = res.tile([ZPAD, F2], f32)
            nc.gpsimd.memset(z2[:], 0)
            nc.sync.dma_start(out=y2_tbl[0:ZPAD, :F2], in_=z2[:])
            nc.sync.dma_start(out=y2_tbl[n_nodes + ZPAD :, :F2], in_=z2[:])

            # ---------------- phase A: rs + Y1 = (Xp @ W1.T) * rs ----------
            moff = 0
            for b in range(nblk):
                s_b = int(SA[b] + SB[b])
                cnt = tmp.tile([P, 1], f32, tag="cnt")
                nc.vector.tensor_reduce(
                    out=cnt[:],
                    in_=msk[:, moff : moff + s_b],
                    axis=mybir.AxisListType.X,
                    op=mybir.AluOpType.add,
                )
                deg = tmp.tile([P, 1], f32, tag="deg")
                nc.vector.tensor_scalar(
                    out=deg[:], in0=cnt[:], scalar1=-1.0, scalar2=1.0,
                    op0=mybir.AluOpType.add, op1=mybir.AluOpType.max,
                )
                inv = tmp.tile([P, 1], f32, tag="inv")
                nc.vector.reciprocal(out=inv[:], in_=deg[:])
                nc.scalar.sqrt(rs_all[:, b : b + 1], inv[:])

                zp = ps.tile([P, F1], f32, tag="z1")
                nc.tensor.matmul(
                    zp[:], lhsT=xpt[:, b * P : (b + 1) * P], rhs=w1t[:],
                    start=True, stop=True,
                )
                y1b = iop.tile([P, F1], bf16, tag="y1b")
                nc.scalar.activation(
                    out=y1b[:], in_=zp[:],
                    func=mybir.ActivationFunctionType.Copy,
                    scale=rs_all[:, b : b + 1],
                )
                rows = min(npc - b * P, P)
                nc.sync.dma_start(
                    out=y1_slice[b * P : b * P + rows, :], in_=y1b[:rows, :]
                )
                moff += s_b

            # ---------------- AllGather Y1 --------------------------------
            nc.gpsimd.collective_compute(
                "AllGather", mybir.AluOpType.bypass,
                replica_groups=[list(range(NCORES))],
                ins=[y1_slice.ap().opt()],
                outs=[y1_gath.ap().opt()],
            )
            nc.sync.dma_start(out=y1_tbl[ZPAD : ZPAD + n_nodes, :F1], in_=y1_gath.ap())

            if stop_after == "A":
                pb = iop.tile([P, F2], bf16, tag="pbb")
                pf = iop.tile([P, F2], f32, tag="pbf")
                for b in range(nblk):
                    rows = min(npc - b * P, P)
                    nc.sync.dma_start(out=pb[:rows, :], in_=y1_tbl[ZPAD + b * P : ZPAD + b * P + rows, :F2])
                    nc.vector.tensor_copy(pf[:rows, :], pb[:rows, :])
                    nc.sync.dma_start(out=out_p[b * P : b * P + rows, :], in_=pf[:rows, :])

            # ---------------- phase C: L1 aggregate + L2 transform --------
            w16 = 0
            for g in (groups if stop_after not in ("A",) else []):
                gsA = int(sum(SA[b] for b in g))
                gsB = int(sum(SB[b] for b in g))
                st1 = stage1p.tile([P, gs_max * F1], bf16, tag="st1")
                for c0, ncol in _calls(gsA):
                    _raw_dma_gather(
                        nc.gpsimd,
                        st1[:, c0 * F1 : (c0 + ncol) * F1].rearrange("p (g f) -> p g f", f=F1),
                        y1_tbl[0:rowsA, :F1],
                        idxw[:, w16 : w16 + 8 * ncol],
                        P * ncol, F1, PITCH1,
                    )
                    w16 += 8 * ncol
                for c0, ncol in _calls(gsB):
                    _raw_dma_gather(
                        nc.gpsimd,
                        st1[:, (gsA + c0) * F1 : (gsA + c0 + ncol) * F1].rearrange("p (g f) -> p g f", f=F1),
                        y1_tbl[half + ZPAD :, :F1],
                        idxw[:, w16 : w16 + 8 * ncol],
                        P * ncol, F1, PITCH1,
                    )
                    w16 += 8 * ncol
                a0, b0 = 0, gsA
                for b in g:
                    _fold(nc, st1, a0, int(SA[b]), F1)
                    _fold(nc, st1, b0, int(SB[b]), F1)
                    nc.vector.tensor_tensor(
                        out=st1[:, a0 * F1 : a0 * F1 + F1],
                        in0=st1[:, a0 * F1 : a0 * F1 + F1],
                        in1=st1[:, b0 * F1 : b0 * F1 + F1],
                        op=mybir.AluOpType.add,
                    )
                    h1 = iop.tile([P, F1], bf16, tag="h1")
                    nc.scalar.activation(
                        out=h1[:], in_=st1[:, a0 * F1 : a0 * F1 + F1],
                        func=mybir.ActivationFunctionType.Relu,
                        scale=rs_all[:, b : b + 1],
                    )
                    trp = ps.tile([F1, P], bf16, tag="tr")
                    nc.tensor.transpose(out=trp[:], in_=h1[:], identity=ident[:])
                    h1t = iop.tile([F1, P], bf16, tag="h1t")
                    nc.vector.tensor_copy(h1t[:], trp[:])
                    z2p = ps.tile([P, F2], f32, tag="z2")
                    nc.tensor.matmul(z2p[:], lhsT=h1t[:], rhs=w2t[:], start=True, stop=True)
                    y2b = iop.tile([P, F2], f32, tag="y2b")
                    nc.scalar.activation(
                        out=y2b[:], in_=z2p[:],
                        func=mybir.ActivationFunctionType.Copy,
                        scale=rs_all[:, b : b + 1],
                    )
                    rows = min(npc - b * P, P)
                    nc.sync.dma_start(
                        out=y2_slice[b * P : b * P + rows, :], in_=y2b[:rows, :]
                    )
                    a0 += int(SA[b])
                    b0 += int(SB[b])

            if stop_after == "C":
                probe = iop.tile([P, F2], f32, tag="probe")
                for b in range(nblk):
                    rows = min(npc - b * P, P)
                    nc.sync.dma_start(out=probe[:rows, :], in_=y2_slice[b * P : b * P + rows, :F2])
                    nc.sync.dma_start(out=out_p[b * P : b * P + rows, :], in_=probe[:rows, :])
            # ---------------- AllGather Y2 --------------------------------
            if stop_after not in ("C", "A"):
                nc.gpsimd.collective_compute(
                    "AllGather", mybir.AluOpType.bypass,
                    replica_groups=[list(range(NCORES))],
                    ins=[y2_slice.ap().opt()],
                    outs=[y2_gath.ap().opt()],
                )
                nc.sync.dma_start(out=y2_tbl[ZPAD : ZPAD + n_nodes, :F2], in_=y2_gath.ap())

            # ---------------- phase E: L2 aggregate -----------------------
            w16 = 0
            for g in (groups if stop_after not in ("C", "A") else []):
                gsA = int(sum(SA[b] for b in g))
                gsB = int(sum(SB[b] for b in g))
                st2 = stage2p.tile([P, gs_max * F2], f32, tag="st2")
                for c0, ncol in _calls(gsA):
                    _raw_dma_gather(
                        nc.gpsimd,
                        st2[:, c0 * F2 : (c0 + ncol) * F2].rearrange("p (g f) -> p g f", f=F2),
                        y2_tbl[0:rowsA, :F2],
                        idxw[:, w16 : w16 + 8 * ncol],
                        P * ncol, F2, PITCH2,
                    )
                    w16 += 8 * ncol
                for c0, ncol in _calls(gsB):
                    _raw_dma_gather(
                        nc.gpsimd,
                        st2[:, (gsA + c0) * F2 : (gsA + c0 + ncol) * F2].rearrange("p (g f) -> p g f", f=F2),
                        y2_tbl[half + ZPAD :, :F2],
                        idxw[:, w16 : w16 + 8 * ncol],
                        P * ncol, F2, PITCH2,
                    )
                    w16 += 8 * ncol
                a0, b0 = 0, gsA
                for b in g:
                    _fold(nc, st2, a0, int(SA[b]), F2)
                    _fold(nc, st2, b0, int(SB[b]), F2)
                    nc.vector.tensor_tensor(
                        out=st2[:, a0 * F2 : a0 * F2 + F2],
                        in0=st2[:, a0 * F2 : a0 * F2 + F2],
                        in1=st2[:, b0 * F2 : b0 * F2 + F2],
                        op=mybir.AluOpType.add,
                    )
                    ob = iop.tile([P, F2], f32, tag="ob")
                    nc.scalar.activation(
                        out=ob[:], in_=st2[:, a0 * F2 : a0 * F2 + F2],
                        func=mybir.ActivationFunctionType.Relu,
                        scale=rs_all[:, b : b + 1],
                    )
                    rows = min(npc - b * P, P)
                    nc.sync.dma_start(
                        out=out_p[b * P : b * P + rows, :], in_=ob[:rows, :]
                    )
                    a0 += int(SA[b])
                    b0 += int(SB[b])

    nc.compile()
    return nc


# ------------------------------------------------------------------- runner
class _Runner:
    """Persistent PJRT executor (mirrors bass2jax.run_bass_via_pjrt but keeps
    the jitted callable for repeated timed runs; no donation — the kernel
    writes every output element)."""

    def __init__(self, nc):
        import jax
        from concourse import bass2jax

        bass2jax.install_neuronx_cc_hook()
        self.nc = nc
        partition_name = nc.partition_id_tensor.name if nc.partition_id_tensor else None
        in_names, out_names, out_avals, zero_outs = [], [], [], []
        for alloc in nc.m.functions[0].allocations:
            if not isinstance(alloc, mybir.MemoryLocationSet):
                continue
            name = alloc.memorylocations[0].name
            if alloc.kind == "ExternalInput":
                if name != partition_name:
                    in_names.append(name)
            elif alloc.kind == "ExternalOutput":
                shape = tuple(alloc.tensor_shape)
                dtype = mybir.dt.np(alloc.dtype)
                out_names.append(name)
                out_avals.append(jax.core.ShapedArray(shape, dtype))
                zero_outs.append(np.zeros(shape, dtype))
        self.in_names, self.out_names = in_names, out_names
        self.zero_outs = zero_outs
        n_params = len(in_names)
        all_in_names = list(in_names) + list(out_names)
        if partition_name is not None:
            all_in_names.append(partition_name)

        def _body(*args):
            operands = list(args)
            if partition_name is not None:
                operands.append(bass2jax.partition_id_tensor())
            outs = bass2jax._bass_exec_p.bind(
                *operands,
                out_avals=tuple(out_avals),
                in_names=tuple(all_in_names),
                out_names=tuple(out_names),
                lowering_input_output_aliases=(),
                sim_require_finite=True,
                sim_require_nnan=True,
                nc=nc,
            )
            return tuple(outs)

        from jax.sharding import Mesh, PartitionSpec, NamedSharding
        from jax.experimental.shard_map import shard_map

        devices = jax.devices()[:NCORES]
        mesh = Mesh(np.asarray(devices), ("core",))
        n_all = n_params + len(out_names)
        self.mesh = mesh
        self.sharding = NamedSharding(mesh, PartitionSpec("core"))
        self.fn = jax.jit(
            shard_map(
                _body, mesh=mesh,
                in_specs=(PartitionSpec("core"),) * n_all,
                out_specs=(PartitionSpec("core"),) * len(out_names),
                check_rep=False,
            ),
            keep_unused=True,
        )
        self.jax = jax

    def prepare(self, in_maps):
        concat = [
            np.concatenate([np.asarray(in_maps[c][k]) for c in range(NCORES)], axis=0)
            for k in self.in_names
        ]
        zeros = [
            np.zeros((NCORES * z.shape[0], *z.shape[1:]), z.dtype)
            for z in self.zero_outs
        ]
        return [self.jax.device_put(a, self.sharding) for a in (*concat, *zeros)]

    def run(self, args):
        outs = self.fn(*args)
        return [np.asarray(o) for o in outs]


def _get_compiled(edges, n_nodes):
    key = (n_nodes, hash(np.asarray(edges).tobytes()))
    if key not in _CACHE:
        st, cores, gperm = _preprocess(edges, n_nodes)
        nc = _build_bass(st)
        _CACHE[key] = (st, cores, gperm, nc, _Runner(nc))
    return _CACHE[key]


def _in_maps(st, cores, gperm, x, W1, W2):
    n_nodes, npc, nblk = st["n_nodes"], st["npc"], st["nblk"]
    x = np.asarray(x, dtype=np.float32)
    w1t = np.ascontiguousarray(np.asarray(W1, np.float32).T)
    w2t = np.ascontiguousarray(np.asarray(W2, np.float32).T).astype(ml_dtypes.bfloat16)
    maps = []
    for c in range(NCORES):
        idxw, msk = cores[c]
        xp = x[gperm[c * npc : (c + 1) * npc]]
        xpt = np.zeros((D, nblk * P), dtype=np.float32)
        xpt[:, :npc] = xp.T
        maps.append({"xpt": xpt, "idxw": idxw, "msk": msk, "w1t": w1t, "w2t": w2t})
    return maps


def kernel(x, edges, W1, W2):
    n_nodes = int(np.asarray(x).shape[0])
    st, cores, gperm, nc, runner = _get_compiled(edges, n_nodes)
    args = runner.prepare(_in_maps(st, cores, gperm, x, W1, W2))
    outs = runner.run(args)
    shard = outs[runner.out_names.index("out")]
    npc = st["npc"]
    full = np.empty((n_nodes, F2), dtype=np.float32)
    full[gperm] = shard.reshape(NCORES * npc, F2)[: n_nodes]
    return full


def bench(x, edges, W1, W2, iters=20):
    n_nodes = int(np.asarray(x).shape[0])
    st, cores, gperm, nc, runner = _get_compiled(edges, n_nodes)
    args = runner.prepare(_in_maps(st, cores, gperm, x, W1, W2))
    for _ in range(3):
        outs = runner.fn(*args)
        self_block = [o.block_until_ready() for o in outs]
    times = []
    for _ in range(iters):
        t0 = time.perf_counter()
        outs = runner.fn(*args)
        [o.block_until_ready() for o in outs]
        times.append(time.perf_counter() - t0)
    return min(times), float(np.median(times))


# revision 14
# speedup vs baseline: 7.0123x; 7.0123x over previous
"""Trainium2 Bass kernel for 2-layer GCN message passing (nn_Method_GCN).

Strategy (8-core SPMD, full inputs in / full output out):
  - Fold weights first: Z = X @ W.T, so per-edge gathers move 64-dim (L1) /
    16-dim (L2) rows instead of 128-dim.
  - Push per-edge symmetric norm into per-node scaling:
        agg_i = rs_i * (sum_{j in N(i)} Y_j + Y_i),  Y = Z * rs,
    rs = deg^-1/2, so messages are plain unweighted row sums.
  - Shard destination nodes across 8 cores (6250 each), degree-sorted within
    each core so 128-node blocks have uniform slot counts.
  - Per block: one dma_gather (InstDMAGatherAnt) per table-half pulls all
    neighbor rows into SBUF [128, S, F]; contiguous pairwise tree-adds on DVE
    reduce slots; per-partition scale+relu on ACT.
  - Tables are AllGather'd (bf16 Y1 / f32 Y2) with 256B row pitch; int16
    gather indices address two halves split at the core 0-3 | 4-7 boundary.
  - deg is computed on device from the slot-validity mask.
"""
import os
import sys
import time

import numpy as np

for _p in ("/opt/trn_rl_repo", "/root/.axon_site/_ro/trn_rl_repo"):
    if os.path.isdir(_p) and _p not in sys.path:
        sys.path.append(_p)

import ml_dtypes
import concourse.bass as bass
import concourse.bacc as bacc
import concourse.tile as tile
import concourse.mybir as mybir

P = 128
NCORES = 8
F1, F2, D = 64, 16, 128
PITCH1 = 128          # bf16 row pitch (256B) in Y1 table
PITCH2 = 64           # f32 row pitch (256B) in Y2 table
ZPAD = 16             # zero rows at head and tail of each table
MAX_GS = 120          # slot budget per gather group (stage tile sizing)

_CACHE = {}


# ----------------------------------------------------------------- raw gather
def _raw_dma_gather(gps, out_ap, in_ap, idxs_ap, num_idxs, elem_size, elem_step):
    """dma_gather without the elem_size%256B assert (HW needs only the row
    pitch to be a 256B multiple — verified on silicon)."""
    nc = gps.bass
    assert idxs_ap.dtype == mybir.dt.int16
    assert in_ap.dtype == out_ap.dtype
    stride_bytes = elem_step * mybir.dt.size(in_ap.dtype)
    assert stride_bytes % 256 == 0
    return gps.add_instruction(
        mybir.InstDMAGatherAnt(
            name=nc.get_next_instruction_name(),
            ins=[
                *gps.lower_ap_dma(in_ap, for_custom_bir_dma=True),
                gps.lower_ap(idxs_ap),
                gps.lower_val_access(gps.to_reg(num_idxs)),
            ],
            outs=[gps.lower_ap(out_ap)],
            transpose=False,
            num_idxs=num_idxs,
            elem_size=elem_size,
            stride_bytes_256=stride_bytes // 256,
            gen_mode=0,
            single_packet=True,
            queue_num=0,
            sbuf_tokens_per_rank=0,
            sbuf_free_dim_per_rank=0,
            sbuf_free_dim_pad_per_rank=0,
            sbuf_byte_offset=0,
        )
    )


def _calls(gs, k=8):
    """Split gs slot columns into per-call spans of <=k columns (<=128*k idx)."""
    out = []
    c0 = 0
    while c0 < gs:
        n = min(k, gs - c0)
        out.append((c0, n))
        c0 += n
    return out


def _fold(nc, stage, col0, width, fdim):
    """In-place pairwise tree-sum of `width` row-chunks of fdim cols starting
    at slot column col0; result lands in slot col0. Slices stay contiguous."""
    w = width
    while w > 1:
        h = w // 2
        lo = col0 * fdim
        nc.vector.tensor_tensor(
            out=stage[:, lo : lo + h * fdim],
            in0=stage[:, lo : lo + h * fdim],
            in1=stage[:, lo + (w - h) * fdim : lo + w * fdim],
            op=mybir.AluOpType.add,
        )
        w = w - h


# ------------------------------------------------------------- preprocessing
def _preprocess(edges, n_nodes):
    """Graph partitioning: per-core degree-sorted node permutation, padded
    per-block slot grids for both table halves, wrapped int16 gather indices,
    validity mask. Returns the SPMD-uniform structure + per-core arrays."""
    npc = n_nodes // NCORES
    nblk = (npc + P - 1) // P
    half = n_nodes // 2

    ed = np.asarray(edges).astype(np.int64, copy=False)
    src, dst = ed[:, 0], ed[:, 1]
    cnt_raw = np.bincount(src, minlength=n_nodes) + np.bincount(dst, minlength=n_nodes)

    # per-core degree-descending permutation
    gperm = np.empty(n_nodes, dtype=np.int64)
    for c in range(NCORES):
        ids = np.arange(c * npc, (c + 1) * npc)
        gperm[c * npc : (c + 1) * npc] = ids[np.argsort(-cnt_raw[ids], kind="stable")]
    gpos = np.empty(n_nodes, dtype=np.int64)
    gpos[gperm] = np.arange(n_nodes)

    # directed messages sorted by destination position; then self-appended
    mdst = gpos[np.concatenate([dst, src])]
    msrc = gpos[np.concatenate([src, dst])]
    order = np.argsort(mdst, kind="stable")
    mdst_s = mdst[order]
    msrc_s = msrc[order]
    bnd = np.searchsorted(mdst_s, np.arange(n_nodes + 1))
    lens = (bnd[1:] - bnd[:-1]).astype(np.int64)          # neighbors per position
    a_of = (msrc_s < half)

    # per-position A/B counts (+ self in its own half)
    nA = np.zeros(n_nodes, dtype=np.int64)
    np.add.at(nA, mdst_s, a_of)
    nB = lens - nA
    selfA = (np.arange(n_nodes) < half)
    nA = nA + selfA
    nB = nB + (~selfA)

    # SPMD-uniform slot counts per block (max over cores), even, >= 2
    SA = np.zeros(nblk, dtype=np.int64)
    SB = np.zeros(nblk, dtype=np.int64)
    for b in range(nblk):
        mA = mB = 0
        for c in range(NCORES):
            g0 = c * npc + b * P
            g1 = min(g0 + P, (c + 1) * npc)
            mA = max(mA, int(nA[g0:g1].max()))
            mB = max(mB, int(nB[g0:g1].max()))
        SA[b] = max(2, (mA + 1) // 2 * 2)
        SB[b] = max(2, (mB + 1) // 2 * 2)

    # group blocks under the stage-slot budget
    groups = []          # list of lists of block ids
    cur, cur_s = [], 0
    for b in range(nblk):
        s = int(SA[b] + SB[b])
        if cur and cur_s + s > MAX_GS:
            groups.append(cur)
            cur, cur_s = [], 0
        cur.append(b)
        cur_s += s
    if cur:
        groups.append(cur)

    gs_max = max(int(sum(SA[b] + SB[b] for b in g)) for g in groups)

    zrA = 0                     # zero row at table head (A-half local)
    zrB = half                  # zero row at table tail (B-half local: ZPAD+half+... )
    tot_s = int((SA + SB).sum())
    tot16 = 8 * tot_s

    # per-core fills
    cores = []
    for c in range(NCORES):
        idxw = np.zeros((16, tot16), dtype=np.int16)
        msk = np.zeros((P, tot_s), dtype=np.float32)
        moff = 0
        w16 = 0
        for g in groups:
            gsA = int(SA[list(g)].sum()) if hasattr(SA, "sum") else 0
            # build A-region and B-region slot grids for this group
            gsA = int(sum(SA[b] for b in g))
            gsB = int(sum(SB[b] for b in g))
            gridA = np.full((P, gsA), zrA, dtype=np.int64)
            gridB = np.full((P, gsB), zrB, dtype=np.int64)
            a0 = b0 = 0
            for b in g:
                g0 = c * npc + b * P
                g1 = min(g0 + P, (c + 1) * npc)
                rows = g1 - g0
                for p in range(rows):
                    gg = g0 + p
                    nb = msrc_s[bnd[gg] : bnd[gg + 1]]
                    nbA = nb[nb < half] + ZPAD
                    nbB = nb[nb >= half] - half
                    if gg < half:
                        nbA = np.append(nbA, gg + ZPAD)
                    else:
                        nbB = np.append(nbB, gg - half)
                    gridA[p, a0 : a0 + len(nbA)] = nbA
                    gridB[p, b0 : b0 + len(nbB)] = nbB
                    tot_real = len(nbA) + len(nbB)
                    msk[p, moff : moff + tot_real] = 1.0
                a0 += int(SA[b])
                b0 += int(SB[b])
                moff += int(SA[b] + SB[b])
            # wrapped int16 lists, one segment per gather call (<=1024 idx):
            # linear i = s*128 + p -> [i%16, i//16]
            for grid, gs in ((gridA, gsA), (gridB, gsB)):
                for c0, ncol in _calls(gs):
                    n = P * ncol
                    lin = grid[:, c0 : c0 + ncol].T.reshape(-1).astype(np.int16)
                    ii = np.arange(n)
                    wrap = np.zeros((16, n // 16), dtype=np.int16)
                    wrap[ii % 16, ii // 16] = lin
                    idxw[:, w16 : w16 + n // 16] = wrap
                    w16 += n // 16
        assert w16 == tot16 and moff == tot_s
        cores.append((np.tile(idxw, (8, 1)), msk))

    struct = {
        "nblk": nblk,
        "npc": npc,
        "half": half,
        "SA": SA,
        "SB": SB,
        "groups": groups,
        "tot_s": tot_s,
        "tot16": tot16,
        "gs_max": gs_max,
        "n_nodes": n_nodes,
    }
    return struct, cores, gperm


# ------------------------------------------------------------------ program
def _build_bass(st, stop_after=None):
    nblk, npc, half = st["nblk"], st["npc"], st["half"]
    n_nodes, tot_s, tot16 = st["n_nodes"], st["tot_s"], st["tot16"]
    SA, SB, groups = st["SA"], st["SB"], st["groups"]
    gs_max = st["gs_max"]
    ncols = nblk * P
    tbl_rows = n_nodes + 2 * ZPAD
    rowsA = half + ZPAD          # A-half view: rows [0, half+ZPAD)
    f32, bf16, i16 = mybir.dt.float32, mybir.dt.bfloat16, mybir.dt.int16

    nc = bacc.Bacc(None, target_bir_lowering=False, debug=False)
    xpt_in = nc.declare_dram_parameter("xpt", [D, ncols], f32, isOutput=False)
    idxw_in = nc.declare_dram_parameter("idxw", [P, tot16], i16, isOutput=False)
    msk_in = nc.declare_dram_parameter("msk", [P, tot_s], f32, isOutput=False)
    w1t_in = nc.declare_dram_parameter("w1t", [D, F1], f32, isOutput=False)
    w2t_in = nc.declare_dram_parameter("w2t", [F1, F2], bf16, isOutput=False)
    out_p = nc.declare_dram_parameter("out", [npc, F2], f32, isOutput=True)

    y1_slice = nc.dram_tensor("y1_slice", [npc, F1], bf16)
    y2_slice = nc.dram_tensor("y2_slice", [npc, F2], f32)
    y1_gath = nc.dram_tensor("y1_gath", [n_nodes, F1], bf16)
    y2_gath = nc.dram_tensor("y2_gath", [n_nodes, F2], f32)
    y1_tbl = nc.dram_tensor("y1_tbl", [tbl_rows, PITCH1], bf16)
    y2_tbl = nc.dram_tensor("y2_tbl", [tbl_rows, PITCH2], f32)

    with tile.TileContext(nc) as tc:
        with (
            tc.tile_pool(name="res", bufs=1) as res,
            tc.tile_pool(name="tmp", bufs=6) as tmp,
            tc.tile_pool(name="stage1", bufs=5) as stage1p,
            tc.tile_pool(name="stage2", bufs=4) as stage2p,
            tc.tile_pool(name="io", bufs=4) as iop,
            tc.tile_pool(name="ps", bufs=2, space="PSUM") as ps,
        ):
            xpt = res.tile([D, ncols], f32)
            nc.sync.dma_start(out=xpt[:], in_=xpt_in.ap())
            idxw = res.tile([P, tot16], i16)
            nc.sync.dma_start(out=idxw[:], in_=idxw_in.ap())
            msk = res.tile([P, tot_s], f32)
            nc.sync.dma_start(out=msk[:], in_=msk_in.ap())
            w1t = res.tile([D, F1], f32)
            nc.sync.dma_start(out=w1t[:], in_=w1t_in.ap())
            w2t = res.tile([F1, F2], bf16)
            nc.sync.dma_start(out=w2t[:], in_=w2t_in.ap())
            ident = res.tile([P, P], bf16)
            from concourse.masks import make_identity

            make_identity(nc, ident[:])
            rs_all = res.tile([P, nblk], f32)

            # zero rows (head+tail) of both tables
            z1 = res.tile([ZPAD, F1], bf16)
            nc.gpsimd.memset(z1[:], 0)
            nc.sync.dma_start(out=y1_tbl[0:ZPAD, :F1], in_=z1[:])
            nc.sync.dma_start(out=y1_tbl[n_nodes + ZPAD :, :F1], in_=z1[:])
            z2 = res.tile([ZPAD, F2], f32)
            nc.gpsimd.memset(z2[:], 0)
            nc.sync.dma_start(out=y2_tbl[0:ZPAD, :F2], in_=z2[:])
            nc.sync.dma_start(out=y2_tbl[n_nodes + ZPAD :, :F2], in_=z2[:])

            # ---------------- phase A: rs + Y1 = (Xp @ W1.T) * rs ----------
            moff = 0
            for b in range(nblk):
                s_b = int(SA[b] + SB[b])
                cnt = tmp.tile([P, 1], f32, tag="cnt")
                nc.vector.tensor_reduce(
                    out=cnt[:],
                    in_=msk[:, moff : moff + s_b],
                    axis=mybir.AxisListType.X,
                    op=mybir.AluOpType.add,
                )
                deg = tmp.tile([P, 1], f32, tag="deg")
                nc.vector.tensor_scalar(
                    out=deg[:], in0=cnt[:], scalar1=-1.0, scalar2=1.0,
                    op0=mybir.AluOpType.add, op1=mybir.AluOpType.max,
                )
                inv = tmp.tile([P, 1], f32, tag="inv")
                nc.vector.reciprocal(out=inv[:], in_=deg[:])
                nc.scalar.sqrt(rs_all[:, b : b + 1], inv[:])

                zp = ps.tile([P, F1], f32, tag="z1")
                nc.tensor.matmul(
                    zp[:], lhsT=xpt[:, b * P : (b + 1) * P], rhs=w1t[:],
                    start=True, stop=True,
                )
                y1b = iop.tile([P, F1], bf16, tag="y1b")
                nc.scalar.activation(
                    out=y1b[:], in_=zp[:],
                    func=mybir.ActivationFunctionType.Copy,
                    scale=rs_all[:, b : b + 1],
                )
                rows = min(npc - b * P, P)
                nc.sync.dma_start(
                    out=y1_slice[b * P : b * P + rows, :], in_=y1b[:rows, :]
                )
                moff += s_b

            # ---------------- AllGather Y1 --------------------------------
            nc.gpsimd.collective_compute(
                "AllGather", mybir.AluOpType.bypass,
                replica_groups=[list(range(NCORES))],
                ins=[y1_slice.ap().opt()],
                outs=[y1_gath.ap().opt()],
            )
            nc.sync.dma_start(out=y1_tbl[ZPAD : ZPAD + n_nodes, :F1], in_=y1_gath.ap())

            if stop_after == "A":
                pb = iop.tile([P, F2], bf16, tag="pbb")
                pf = iop.tile([P, F2], f32, tag="pbf")
                for b in range(nblk):
                    rows = min(npc - b * P, P)
                    nc.sync.dma_start(out=pb[:rows, :], in_=y1_tbl[ZPAD + b * P : ZPAD + b * P + rows, :F2])
                    nc.vector.tensor_copy(pf[:rows, :], pb[:rows, :])
                    nc.sync.dma_start(out=out_p[b * P : b * P + rows, :], in_=pf[:rows, :])

            # ---------------- phase C: L1 aggregate + L2 transform --------
            w16 = 0
            for g in (groups if stop_after not in ("A",) else []):
                gsA = int(sum(SA[b] for b in g))
                gsB = int(sum(SB[b] for b in g))
                st1 = stage1p.tile([P, gs_max * F1], bf16, tag="st1")
                for c0, ncol in _calls(gsA):
                    _raw_dma_gather(
                        nc.gpsimd,
                        st1[:, c0 * F1 : (c0 + ncol) * F1].rearrange("p (g f) -> p g f", f=F1),
                        y1_tbl[0:rowsA, :F1],
                        idxw[:, w16 : w16 + 8 * ncol],
                        P * ncol, F1, PITCH1,
                    )
                    w16 += 8 * ncol
                for c0, ncol in _calls(gsB):
                    _raw_dma_gather(
                        nc.gpsimd,
                        st1[:, (gsA + c0) * F1 : (gsA + c0 + ncol) * F1].rearrange("p (g f) -> p g f", f=F1),
                        y1_tbl[half + ZPAD :, :F1],
                        idxw[:, w16 : w16 + 8 * ncol],
                        P * ncol, F1, PITCH1,
                    )
                    w16 += 8 * ncol
                a0, b0 = 0, gsA
                for b in g:
                    _fold(nc, st1, a0, int(SA[b]), F1)
                    _fold(nc, st1, b0, int(SB[b]), F1)
                    nc.vector.tensor_tensor(
                        out=st1[:, a0 * F1 : a0 * F1 + F1],
                        in0=st1[:, a0 * F1 : a0 * F1 + F1],
                        in1=st1[:, b0 * F1 : b0 * F1 + F1],
                        op=mybir.AluOpType.add,
                    )
                    h1 = iop.tile([P, F1], bf16, tag="h1")
                    nc.scalar.activation(
                        out=h1[:], in_=st1[:, a0 * F1 : a0 * F1 + F1],
                        func=mybir.ActivationFunctionType.Relu,
                        scale=rs_all[:, b : b + 1],
                    )
                    trp = ps.tile([F1, P], bf16, tag="tr")
                    nc.tensor.transpose(out=trp[:], in_=h1[:], identity=ident[:])
                    h1t = iop.tile([F1, P], bf16, tag="h1t")
                    nc.vector.tensor_copy(h1t[:], trp[:])
                    z2p = ps.tile([P, F2], f32, tag="z2")
                    nc.tensor.matmul(z2p[:], lhsT=h1t[:], rhs=w2t[:], start=True, stop=True)
                    y2b = iop.tile([P, F2], f32, tag="y2b")
                    nc.scalar.activation(
                        out=y2b[:], in_=z2p[:],
                        func=mybir.ActivationFunctionType.Copy,
                        scale=rs_all[:, b : b + 1],
                    )
                    rows = min(npc - b * P, P)
                    nc.sync.dma_start(
                        out=y2_slice[b * P : b * P + rows, :], in_=y2b[:rows, :]
                    )
                    a0 += int(SA[b])
                    b0 += int(SB[b])

            if stop_after == "C":
                probe = iop.tile([P, F2], f32, tag="probe")
                for b in range(nblk):
                    rows = min(npc - b * P, P)
                    nc.sync.dma_start(out=probe[:rows, :], in_=y2_slice[b * P : b * P + rows, :F2])
                    nc.sync.dma_start(out=out_p[b * P : b * P + rows, :], in_=probe[:rows, :])
            # ---------------- AllGather Y2 --------------------------------
            if stop_after not in ("C", "A"):
                nc.gpsimd.collective_compute(
                    "AllGather", mybir.AluOpType.bypass,
                    replica_groups=[list(range(NCORES))],
                    ins=[y2_slice.ap().opt()],
                    outs=[y2_gath.ap().opt()],
                )
                nc.sync.dma_start(out=y2_tbl[ZPAD : ZPAD + n_nodes, :F2], in_=y2_gath.ap())

            # ---------------- phase E: L2 aggregate -----------------------
            w16 = 0
            for g in (groups if stop_after not in ("C", "A") else []):
                gsA = int(sum(SA[b] for b in g))
                gsB = int(sum(SB[b] for b in g))
                st2 = stage2p.tile([P, gs_max * F2], f32, tag="st2")
                for c0, ncol in _calls(gsA):
                    _raw_dma_gather(
                        nc.gpsimd,
                        st2[:, c0 * F2 : (c0 + ncol) * F2].rearrange("p (g f) -> p g f", f=F2),
                        y2_tbl[0:rowsA, :F2],
                        idxw[:, w16 : w16 + 8 * ncol],
                        P * ncol, F2, PITCH2,
                    )
                    w16 += 8 * ncol
                for c0, ncol in _calls(gsB):
                    _raw_dma_gather(
                        nc.gpsimd,
                        st2[:, (gsA + c0) * F2 : (gsA + c0 + ncol) * F2].rearrange("p (g f) -> p g f", f=F2),
                        y2_tbl[half + ZPAD :, :F2],
                        idxw[:, w16 : w16 + 8 * ncol],
                        P * ncol, F2, PITCH2,
                    )
                    w16 += 8 * ncol
                a0, b0 = 0, gsA
                for b in g:
                    _fold(nc, st2, a0, int(SA[b]), F2)
                    _fold(nc, st2, b0, int(SB[b]), F2)
                    nc.vector.tensor_tensor(
                        out=st2[:, a0 * F2 : a0 * F2 + F2],
                        in0=st2[:, a0 * F2 : a0 * F2 + F2],
                        in1=st2[:, b0 * F2 : b0 * F2 + F2],
                        op=mybir.AluOpType.add,
                    )
                    ob = iop.tile([P, F2], f32, tag="ob")
                    nc.scalar.activation(
                        out=ob[:], in_=st2[:, a0 * F2 : a0 * F2 + F2],
                        func=mybir.ActivationFunctionType.Relu,
                        scale=rs_all[:, b : b + 1],
                    )
                    rows = min(npc - b * P, P)
                    nc.sync.dma_start(
                        out=out_p[b * P : b * P + rows, :], in_=ob[:rows, :]
                    )
                    a0 += int(SA[b])
                    b0 += int(SB[b])

    nc.compile()
    return nc


# ------------------------------------------------------------------- runner
class _Runner:
    """Persistent PJRT executor (mirrors bass2jax.run_bass_via_pjrt but keeps
    the jitted callable for repeated timed runs; no donation — the kernel
    writes every output element)."""

    def __init__(self, nc):
        import jax
        from concourse import bass2jax

        bass2jax.install_neuronx_cc_hook()
        self.nc = nc
        partition_name = nc.partition_id_tensor.name if nc.partition_id_tensor else None
        in_names, out_names, out_avals, zero_outs = [], [], [], []
        for alloc in nc.m.functions[0].allocations:
            if not isinstance(alloc, mybir.MemoryLocationSet):
                continue
            name = alloc.memorylocations[0].name
            if alloc.kind == "ExternalInput":
                if name != partition_name:
                    in_names.append(name)
            elif alloc.kind == "ExternalOutput":
                shape = tuple(alloc.tensor_shape)
                dtype = mybir.dt.np(alloc.dtype)
                out_names.append(name)
                out_avals.append(jax.core.ShapedArray(shape, dtype))
                zero_outs.append(np.zeros(shape, dtype))
        self.in_names, self.out_names = in_names, out_names
        self.zero_outs = zero_outs
        n_params = len(in_names)
        all_in_names = list(in_names) + list(out_names)
        if partition_name is not None:
            all_in_names.append(partition_name)

        def _body(*args):
            operands = list(args)
            if partition_name is not None:
                operands.append(bass2jax.partition_id_tensor())
            outs = bass2jax._bass_exec_p.bind(
                *operands,
                out_avals=tuple(out_avals),
                in_names=tuple(all_in_names),
                out_names=tuple(out_names),
                lowering_input_output_aliases=(),
                sim_require_finite=True,
                sim_require_nnan=True,
                nc=nc,
            )
            return tuple(outs)

        from jax.sharding import Mesh, PartitionSpec, NamedSharding
        from jax.experimental.shard_map import shard_map

        devices = jax.devices()[:NCORES]
        mesh = Mesh(np.asarray(devices), ("core",))
        n_all = n_params + len(out_names)
        self.mesh = mesh
        self.sharding = NamedSharding(mesh, PartitionSpec("core"))
        self.fn = jax.jit(
            shard_map(
                _body, mesh=mesh,
                in_specs=(PartitionSpec("core"),) * n_all,
                out_specs=(PartitionSpec("core"),) * len(out_names),
                check_rep=False,
            ),
            keep_unused=True,
        )
        self.jax = jax

    def prepare(self, in_maps):
        concat = [
            np.concatenate([np.asarray(in_maps[c][k]) for c in range(NCORES)], axis=0)
            for k in self.in_names
        ]
        zeros = [
            np.zeros((NCORES * z.shape[0], *z.shape[1:]), z.dtype)
            for z in self.zero_outs
        ]
        return [self.jax.device_put(a, self.sharding) for a in (*concat, *zeros)]

    def run(self, args):
        outs = self.fn(*args)
        return [np.asarray(o) for o in outs]


def _get_compiled(edges, n_nodes):
    key = (n_nodes, hash(np.asarray(edges).tobytes()))
    if key not in _CACHE:
        st, cores, gperm = _preprocess(edges, n_nodes)
        nc = _build_bass(st)
        _CACHE[key] = (st, cores, gperm, nc, _Runner(nc))
    return _CACHE[key]


def _in_maps(st, cores, gperm, x, W1, W2):
    n_nodes, npc, nblk = st["n_nodes"], st["npc"], st["nblk"]
    x = np.asarray(x, dtype=np.float32)
    w1t = np.ascontiguousarray(np.asarray(W1, np.float32).T)
    w2t = np.ascontiguousarray(np.asarray(W2, np.float32).T).astype(ml_dtypes.bfloat16)
    maps = []
    for c in range(NCORES):
        idxw, msk = cores[c]
        xp = x[gperm[c * npc : (c + 1) * npc]]
        xpt = np.zeros((D, nblk * P), dtype=np.float32)
        xpt[:, :npc] = xp.T
        maps.append({"xpt": xpt, "idxw": idxw, "msk": msk, "w1t": w1t, "w2t": w2t})
    return maps


def kernel(x, edges, W1, W2):
    n_nodes = int(np.asarray(x).shape[0])
    st, cores, gperm, nc, runner = _get_compiled(edges, n_nodes)
    args = runner.prepare(_in_maps(st, cores, gperm, x, W1, W2))
    outs = runner.run(args)
    shard = outs[runner.out_names.index("out")]
    npc = st["npc"]
    full = np.empty((n_nodes, F2), dtype=np.float32)
    full[gperm] = shard.reshape(NCORES * npc, F2)[: n_nodes]
    return full


def bench(x, edges, W1, W2, iters=20):
    n_nodes = int(np.asarray(x).shape[0])
    st, cores, gperm, nc, runner = _get_compiled(edges, n_nodes)
    args = runner.prepare(_in_maps(st, cores, gperm, x, W1, W2))
    for _ in range(3):
        outs = runner.fn(*args)
        self_block = [o.block_until_ready() for o in outs]
    times = []
    for _ in range(iters):
        t0 = time.perf_counter()
        outs = runner.fn(*args)
        [o.block_until_ready() for o in outs]
        times.append(time.perf_counter() - t0)
    return min(times), float(np.median(times))
